# revision 1
# baseline (speedup 1.0000x reference)
"""KronEmbedding lookup kernel for 8 TRN2 NeuronCores.

Math: w = einsum('sia,sjb->ijab', A, B).reshape(50176, 2048); out = w[x].
Never materializes w. Per token t with i=x//224, j=x%224:
    out[t] = sum_s outer(A[s,i,:], B[s,j,:])   -> (64*32 = 2048 floats)

Strategy (data-parallel over tokens, 1024 tokens/core):
- Host: repack A -> A3[8i+s, a] (rows 256B), B -> B3[8j+s, b-padded-to-64],
  and build per-(token,s) gather indices in the SWDGE wrapped-int16 layout.
- Device per 128-token tile:
    dma_gather A-rows -> stacked lhsT layout [(8k+s)%128, group, 64]
    dma_gather B-rows -> same layout
    round fp32 -> fp32r (DVE copy) for full-rate PE matmuls
    16 strided SBUF->SBUF DMAs scatter B rows into a block-diagonal moving
      operand BD[(8k+s), (k,b)] (off-diag zeros persist across tiles)
    per 16-token group: matmul(out[a, (k,b)] = Ag_stacked^T @ BD), two
      groups packed per PSUM tile on partition halves
    evacuate PSUM -> SBUF (DVE/ACT alternating), DMA 256KB blocks to HBM
- Host: reorder device-native [tile, pair, 128, 512] blocks to token-major.
"""
import numpy as np
from contextlib import ExitStack

import concourse.bass as bass
import concourse.bacc as bacc
import concourse.tile as tile
import concourse.mybir as mybir
from concourse import bass_utils

dt = mybir.dt

R, M1, N1, M2, N2 = 8, 224, 64, 224, 32
VOCAB, EMB = M1 * M2, N1 * N2          # 50176, 2048
BATCH, SEQ = 4, 2048
NTOK = BATCH * SEQ                     # 8192
NCORES = 8
TPC = NTOK // NCORES                   # 1024 tokens per core
NTILES = TPC // 128                    # 8 tiles of 128 tokens
NGRP = 8                               # 16-token groups per tile

_CACHE = {}


def _build():
    nc = bacc.Bacc("TRN2", num_devices=NCORES)
    A3 = nc.dram_tensor("A3", [M1 * R, 64], dt.float32, kind="ExternalInput")
    B3 = nc.dram_tensor("B3", [M2 * R, 64], dt.float32, kind="ExternalInput")
    idxA = nc.dram_tensor("idxA", [128, TPC * 8 // 16], dt.int16, kind="ExternalInput")
    idxB = nc.dram_tensor("idxB", [128, TPC * 8 // 16], dt.int16, kind="ExternalInput")
    out = nc.dram_tensor("out", [NTILES, 4, 64, 1024], dt.float32, kind="ExternalOutput")

    with tile.TileContext(nc) as tc, ExitStack() as ctx:
        const_pool = ctx.enter_context(tc.tile_pool(name="const", bufs=1))
        agf_pool = ctx.enter_context(tc.tile_pool(name="agf", bufs=3))
        bgf_pool = ctx.enter_context(tc.tile_pool(name="bgf", bufs=3))
        agr_pool = ctx.enter_context(tc.tile_pool(name="agr", bufs=3))
        bgr_pool = ctx.enter_context(tc.tile_pool(name="bgr", bufs=3))
        ev_pool = ctx.enter_context(tc.tile_pool(name="ev", bufs=6))
        ps_pool = ctx.enter_context(tc.tile_pool(name="ps", bufs=3, space="PSUM"))

        idxA_sb = const_pool.tile([128, 512], dt.int16, tag="idxA")
        idxB_sb = const_pool.tile([128, 512], dt.int16, tag="idxB")
        nc.sync.dma_start(idxA_sb[:], idxA[:])
        nc.sync.dma_start(idxB_sb[:], idxB[:])

        # Two persistent block-diagonal buffers (double buffer by hand so the
        # off-diagonal zeros are written exactly once).
        bd_bufs = [
            const_pool.tile([128, NGRP, 512], dt.float32r, tag=f"bd{i}", name=f"bd{i}")
            for i in range(2)
        ]
        for b in bd_bufs:
            nc.gpsimd.memset(b[:].bitcast(dt.float32), 0.0)

        for t in range(NTILES):
            agf = agf_pool.tile([128, NGRP, 64], dt.float32, tag="agf")
            nc.gpsimd.dma_gather(
                agf[:], A3[:], idxA_sb[:, 64 * t:64 * (t + 1)], 1024, 1024, 64
            )
            bgf = bgf_pool.tile([128, NGRP, 64], dt.float32, tag="bgf")
            nc.gpsimd.dma_gather(
                bgf[:], B3[:], idxB_sb[:, 64 * t:64 * (t + 1)], 1024, 1024, 64
            )
            agr = agr_pool.tile([128, NGRP, 64], dt.float32r, tag="agr")
            nc.vector.tensor_copy(agr[:], agf[:])
            bgr = bgr_pool.tile([128, NGRP, 64], dt.float32r, tag="bgr")
            nc.vector.tensor_copy(bgr[:], bgf[:])

            bd = bd_bufs[t % 2]
            for k in range(16):
                nc.sync.dma_start(
                    bd[8 * k:8 * k + 8, :, 32 * k:32 * k + 32],
                    bgr[8 * k:8 * k + 8, :, 0:32],
                )

            for pair in range(4):
                ps = ps_pool.tile([64, 1024], dt.float32, tag="ps")
                for h in range(2):
                    g = 2 * pair + h
                    nc.tensor.matmul(
                        ps[:, 512 * h:512 * h + 512],
                        agr[:, g, :],
                        bd[:, g, :],
                        start=True,
                        stop=True,
                    )
                ev = ev_pool.tile([64, 1024], dt.float32, tag="ev")
                if pair % 2 == 0:
                    nc.vector.tensor_copy(ev[:], ps[:])
                else:
                    nc.scalar.copy(ev[:], ps[:])
                nc.sync.dma_start(out[t, pair], ev[:])

    nc.compile()
    return nc


def _wrap_idxs(idx: np.ndarray) -> np.ndarray:
    """[n] -> SWDGE wrapped layout [128, n//16] int16 (16-wrap, 8x replicated)."""
    n = idx.shape[0]
    w = idx.reshape(n // 16, 16).T.astype(np.int16)
    return np.ascontiguousarray(np.tile(w, (8, 1)))


def kernel(A: np.ndarray, B: np.ndarray, x: np.ndarray) -> np.ndarray:
    A = np.asarray(A, dtype=np.float32)
    B = np.asarray(B, dtype=np.float32)
    xl = np.asarray(x).astype(np.int64).reshape(-1)           # [8192]

    A3 = np.ascontiguousarray(A.transpose(1, 0, 2).reshape(M1 * R, 64))
    B3 = np.zeros((M2 * R, 64), dtype=np.float32)
    B3[:, :32] = B.transpose(1, 0, 2).reshape(M2 * R, 32)

    i_all = (xl // M2).astype(np.int64)
    j_all = (xl % M2).astype(np.int64)

    if "nc" not in _CACHE:
        _CACHE["nc"] = _build()
    nc = _CACHE["nc"]

    s = np.arange(R, dtype=np.int64)
    in_maps = []
    for c in range(NCORES):
        sl = slice(c * TPC, (c + 1) * TPC)
        ia = (i_all[sl, None] * R + s[None, :]).reshape(-1)   # [8192] per core
        jb = (j_all[sl, None] * R + s[None, :]).reshape(-1)
        in_maps.append(
            dict(A3=A3, B3=B3, idxA=_wrap_idxs(ia), idxB=_wrap_idxs(jb))
        )

    res = bass_utils.run_bass_kernel_spmd(nc, in_maps, core_ids=list(range(NCORES)))

    outs = []
    for c in range(NCORES):
        o = res.results[c]["out"]                      # [8, 4, 128, 512]
        o = o.reshape(NTILES, 4, 64, 2, 16, 32)        # [t, p, a, gh, k, b]
        o = o.transpose(0, 1, 3, 4, 2, 5)              # [t, p, gh, k, a, b]
        outs.append(o.reshape(TPC, EMB))
    full = np.concatenate(outs, axis=0)                # [8192, 2048]
    return full.reshape(BATCH, SEQ, EMB)



# revision 6
# speedup vs baseline: 2.4826x; 2.4826x over previous
"""KronEmbedding lookup kernel for 8 TRN2 NeuronCores.

Math: w = einsum('sia,sjb->ijab', A, B).reshape(50176, 2048); out = w[x].
Never materializes w. Per token t with i=x//224, j=x%224:
    out[t] = sum_s outer(A[s,i,:], B[s,j,:])   -> (64*32 = 2048 floats)

Strategy (data-parallel over tokens, 1024 tokens/core, all-bf16 device
compute; tolerance 2e-2 >> bf16 rounding):
- Host: token-major bf16 tables A4[i] = A[:,i,:] (512 vals, s-major) and
  B4[j] = B[:,j,:] (256 vals); per-core idx arrays in SWDGE wrapped int16.
- Device per core:
  * 2+2 SWDGE gathers, ONE ROW PER TOKEN (8x fewer Q7 row descriptors
    than per-(token,s) gathers; gather cost is per-row, not per-byte).
    Half h covers tiles 4h..4h+4: Ahat [128 t, 4 c, 512], Bhat [.., 256].
  * 8 shuffle DMAs per (operand, half) - one per s - move the WHOLE half
    from token-major to contraction layout Ag[(16s+k) part, g, c, 64]
    (token t = 8k+g within tile c). src [16k, 8g, 4c, 64] / dst merge to
    3 lowered dims; ~0.6us per DMA on the HWDGE sequencer.
  * Block-diag moving operand via DVE broadcast x static 0/1 mask:
    BD[p, (k',b)] = Bg[p, g, c, b] * (k' == p%16). No per-piece
    descriptors anywhere.
  * Per (tile c, group g): matmul psum[64,512] = Ag[:,g,c,:]^T @ BD.
    4 groups packed per [128,1024] psum tile (partition halves x column
    halves), 2 psum tiles per tile c.
  * Evac psum -> bf16 SBUF (ACT/DVE), one 512 KB out-DMA per tile.
- Host: upcast bf16 -> f32, unshuffle token/emb order.
"""
import numpy as np
import ml_dtypes
from contextlib import ExitStack

import concourse.bass as bass
import concourse.bacc as bacc
import concourse.tile as tile
import concourse.mybir as mybir
from concourse import bass_utils

dt = mybir.dt
bf16 = ml_dtypes.bfloat16

R, M1, N1, M2, N2 = 8, 224, 64, 224, 32
VOCAB, EMB = M1 * M2, N1 * N2          # 50176, 2048
BATCH, SEQ = 4, 2048
NTOK = BATCH * SEQ                     # 8192
NCORES = 8
TPC = NTOK // NCORES                   # 1024 tokens per core
NTILES = TPC // 128                    # 8 tiles of 128 tokens
NG = 8                                 # groups per tile (token t = 8k+g)

_CACHE = {}


def _build():
    nc = bacc.Bacc("TRN2", num_devices=NCORES)
    A4 = nc.dram_tensor("A4", [M1, 512], dt.bfloat16, kind="ExternalInput")
    B4 = nc.dram_tensor("B4", [M2, 256], dt.bfloat16, kind="ExternalInput")
    idxA = nc.dram_tensor("idxA", [128, 64], dt.int16, kind="ExternalInput")
    idxB = nc.dram_tensor("idxB", [128, 64], dt.int16, kind="ExternalInput")
    maskT = nc.dram_tensor("maskT", [128, 512], dt.bfloat16, kind="ExternalInput")
    # DRAM scratch for the partition-shuffle round trip (host ignores)
    Asc = nc.dram_tensor("Asc", [8, 16, 8, 8, 64], dt.bfloat16, kind="ExternalOutput")
    Bsc = nc.dram_tensor("Bsc", [8, 16, 8, 8, 32], dt.bfloat16, kind="ExternalOutput")
    out = nc.dram_tensor(
        "out", [NTILES, 128, 2048], dt.bfloat16, kind="ExternalOutput"
    )

    with tile.TileContext(nc) as tc, ExitStack() as ctx:
        cpool = ctx.enter_context(tc.tile_pool(name="const", bufs=1))
        ahp = ctx.enter_context(tc.tile_pool(name="ah", bufs=1))
        bhp = ctx.enter_context(tc.tile_pool(name="bh", bufs=1))
        agp = ctx.enter_context(tc.tile_pool(name="ag", bufs=1))
        bgp = ctx.enter_context(tc.tile_pool(name="bg", bufs=1))
        bdp = ctx.enter_context(tc.tile_pool(name="bd", bufs=6))
        psp = ctx.enter_context(tc.tile_pool(name="ps", bufs=4, space="PSUM"))
        evp = ctx.enter_context(tc.tile_pool(name="ev", bufs=3))

        idxA_sb = cpool.tile([128, 64], dt.int16, tag="idxA")
        idxB_sb = cpool.tile([128, 64], dt.int16, tag="idxB")
        mask_sb = cpool.tile([128, 512], dt.bfloat16, tag="mask")
        nc.sync.dma_start(idxA_sb[:], idxA[:])
        nc.sync.dma_start(idxB_sb[:], idxB[:])
        nc.sync.dma_start(mask_sb[:], maskT[:])

        # token-major gathers: row idx[c*128+p] lands at out[p, c]
        ah = ahp.tile([128, 8, 512], dt.bfloat16, tag="ah")
        nc.gpsimd.dma_gather(ah[:], A4[:], idxA_sb[:], 1024, 1024, 512)
        bh = bhp.tile([128, 8, 256], dt.bfloat16, tag="bh")
        nc.gpsimd.dma_gather(bh[:], B4[:], idxB_sb[:], 1024, 1024, 256)

        # shuffle via DRAM round trip: token t = 8k+g of tile c; 8 writes
        # (s-block of every token) + 1 full readback per operand.
        # Asc[s,k,g,c,.] address = linear in (p=8k+g) on the write side.
        for s in range(8):
            nc.sync.dma_start(
                Asc[s].rearrange("k g c a -> (k g) (c a)"),
                ah[:, :, 64 * s:64 * s + 64],
            )
            nc.scalar.dma_start(
                Bsc[s].rearrange("k g c a -> (k g) (c a)"),
                bh[:, :, 32 * s:32 * s + 32],
            )
        ag = agp.tile([128, NG, NTILES, 64], dt.bfloat16, tag="ag")
        bg = bgp.tile([128, NG, NTILES, 32], dt.bfloat16, tag="bg")
        nc.sync.dma_start(ag[:], Asc[:].rearrange("s k g c a -> (s k) g c a"))
        nc.scalar.dma_start(bg[:], Bsc[:].rearrange("s k g c a -> (s k) g c a"))

        # per tile: 8 BD builds (DVE) + 8 matmuls; evac ACT/DVE; out DMA.
        for c in range(NTILES):
            ev = evp.tile([128, 2, 1024], dt.bfloat16, tag="ev")
            for half in range(2):
                ps = psp.tile([128, 1024], dt.float32, tag="ps")
                for q in range(4):
                    g = 4 * half + q
                    bdt = bdp.tile([128, 16, 32], dt.bfloat16, tag="bd")
                    src = (
                        bg[:, g, c, :]
                        .unsqueeze(1)
                        .broadcast_to([128, 16, 32])
                    )
                    nc.vector.tensor_mul(
                        bdt[:], src, mask_sb[:].rearrange("p (k b) -> p k b", k=16)
                    )
                    nc.tensor.matmul(
                        ps[64 * (q % 2):64 * (q % 2) + 64,
                           512 * (q // 2):512 * (q // 2) + 512],
                        ag[:, g, c, :],
                        bdt[:].rearrange("p k b -> p (k b)"),
                        start=True,
                        stop=True,
                    )
                if half == 0:
                    nc.scalar.copy(ev[:, half, :], ps[:])
                else:
                    nc.vector.tensor_copy(ev[:, half, :], ps[:])
            nc.scalar.dma_start(out[c], ev[:].rearrange("p h e -> p (h e)"))

    nc.compile()
    return nc


def _wrap_idxs(idx: np.ndarray) -> np.ndarray:
    """[n] -> SWDGE wrapped layout [128, n//16] int16; gather places
    row idx[c*128+p] at out[p, c]."""
    n = idx.shape[0]
    w = idx.reshape(n // 16, 16).T.astype(np.int16)
    return np.ascontiguousarray(np.tile(w, (8, 1)))


def _in_maps(A, B, x):
    A = np.asarray(A, dtype=np.float32)
    B = np.asarray(B, dtype=np.float32)
    xl = np.asarray(x).astype(np.int64).reshape(-1)           # [8192]

    A4 = np.ascontiguousarray(A.transpose(1, 0, 2).reshape(M1, 512)).astype(bf16)
    B4 = np.ascontiguousarray(B.transpose(1, 0, 2).reshape(M2, 256)).astype(bf16)

    i_all = (xl // M2).astype(np.int64)
    j_all = (xl % M2).astype(np.int64)

    # maskT[p, k'*32+b] = (k' == p % 16)
    mask = (np.arange(16)[None, :, None] == (np.arange(128) % 16)[:, None, None])
    maskT = np.ascontiguousarray(
        np.broadcast_to(mask, (128, 16, 32)).reshape(128, 512).astype(bf16)
    )

    in_maps = []
    for core in range(NCORES):
        sl = slice(core * TPC, (core + 1) * TPC)
        in_maps.append(
            dict(
                A4=A4,
                B4=B4,
                idxA=_wrap_idxs(i_all[sl]),
                idxB=_wrap_idxs(j_all[sl]),
                maskT=maskT,
            )
        )
    return in_maps


def _decode(res):
    outs = []
    for core in range(NCORES):
        o = np.asarray(res.results[core]["out"]).astype(np.float32)
        # out[c, p, inner]: p = 64*rowhalf + a (rowhalf = q%2);
        # inner = 1024*half + 512*colblk + 32*k + b  (g = 4*half+2*colblk+rowhalf)
        o = o.reshape(NTILES, 2, 64, 2, 2, 16, 32)  # c, rh, a, half, cb, k, b
        o = o.transpose(0, 5, 3, 4, 1, 2, 6)        # c, k, half, cb, rh, a, b
        outs.append(o.reshape(TPC, EMB))            # token = c*128 + 8k + g
    full = np.concatenate(outs, axis=0)             # [8192, 2048]
    return full.reshape(BATCH, SEQ, EMB).astype(np.float32)


def kernel(A: np.ndarray, B: np.ndarray, x: np.ndarray) -> np.ndarray:
    if "nc" not in _CACHE:
        _CACHE["nc"] = _build()
    nc = _CACHE["nc"]
    in_maps = _in_maps(A, B, x)
    res = bass_utils.run_bass_kernel_spmd(nc, in_maps, core_ids=list(range(NCORES)))
    return _decode(res)
